# revision 2
# baseline (speedup 1.0000x reference)
"""CodeSage attention (B=2, S=2048, H=1024, 16 heads x 64) on 8 Trainium2 cores.

Sharding: tensor-parallel over heads — 2 heads per core. Each core computes
its head-group's QKV projection, attention, and the c_proj partial product;
the host sums the 8 partials and adds c_proj_b.

Device-side layout (per core, bf16 matmuls / fp32 accumulation):
  phase 1:  qT,kT [128=2*64, 4096] = W_slice^T @ hs^T   (hsT supplied by host)
            v     [4096, 128] natural
  phase 2:  per (batch, 512-query-block), flash-style with transposed scores:
            scoresT[sk,sq] = kT^T-slices @ qT  (2-head row-tiled PE packing)
            probs = exp(scoresT + mask)        (ScalarE, mask as per-partition bias)
            ctxT  [128, sq] and sumexp (ones-matmul, 64x replicated) via
            col-tiled PE packing; normalize on DVE; c_proj matmul; DMA out.
"""

import numpy as np
import ml_dtypes

B, S, H = 2, 2048, 1024
NH, HD = 16, 64
NCORES = 8
HPC = NH // NCORES          # heads per core = 2
DC = HPC * HD               # per-core head dims = 128
T = B * S                   # 4096 tokens
KC = H // 128               # 8 contraction chunks
NBLK = T // 512             # 8 column blocks of 512 tokens
SQB = S // 512              # 4 query blocks per batch
SKT = S // 128              # 16 key tiles per batch

_CACHE = {}


def _build_nc():
    import concourse.mybir as mybir
    import concourse.tile as tile
    from concourse import bacc

    f32 = mybir.dt.float32
    bf16 = mybir.dt.bfloat16

    nc = bacc.Bacc("TRN2", target_bir_lowering=False, debug=False,
                   num_devices=NCORES)

    hsT_d = nc.dram_tensor("hsT", [H, T], bf16, kind="ExternalInput")
    wq_d = nc.dram_tensor("wq", [H, DC], bf16, kind="ExternalInput")
    wk_d = nc.dram_tensor("wk", [H, DC], bf16, kind="ExternalInput")
    wv_d = nc.dram_tensor("wv", [H, DC], bf16, kind="ExternalInput")
    wp_d = nc.dram_tensor("wp", [DC, H], bf16, kind="ExternalInput")
    bq_d = nc.dram_tensor("bq", [DC, 1], f32, kind="ExternalInput")
    bk_d = nc.dram_tensor("bk", [DC, 1], f32, kind="ExternalInput")
    bv_d = nc.dram_tensor("bv", [1, DC], bf16, kind="ExternalInput")
    mask_d = nc.dram_tensor("mask", [B, S], f32, kind="ExternalInput")
    out_d = nc.dram_tensor("out", [T, H], f32, kind="ExternalOutput")

    EXP = mybir.ActivationFunctionType.Exp
    MULT = mybir.AluOpType.mult
    ADD = mybir.AluOpType.add

    with tile.TileContext(nc) as tc:
        with (
            tc.tile_pool(name="const", bufs=1) as cpool,
            tc.tile_pool(name="qkv", bufs=1) as qpool,
        ):
            # --- persistent SBUF tensors -----------------------------------
            wq_sb = cpool.tile([128, KC, DC], bf16)
            wk_sb = cpool.tile([128, KC, DC], bf16)
            wv_sb = cpool.tile([128, KC, DC], bf16)
            wp_sb = cpool.tile([DC, H], bf16)
            bq_sb = cpool.tile([DC, 1], f32)
            bk_sb = cpool.tile([DC, 1], f32)
            bvr_sb = cpool.tile([1, DC], bf16)
            ones1 = cpool.tile([1, 128], bf16)
            ones64 = cpool.tile([128, 64], bf16)
            mask_sb = cpool.tile([128, B, SKT], f32)
            bv_bc = cpool.tile([128, DC], f32)

            nc.sync.dma_start(wq_sb[:], wq_d.ap().rearrange("(k p) m -> p k m", p=128))
            nc.sync.dma_start(wk_sb[:], wk_d.ap().rearrange("(k p) m -> p k m", p=128))
            nc.sync.dma_start(wv_sb[:], wv_d.ap().rearrange("(k p) m -> p k m", p=128))
            nc.sync.dma_start(wp_sb[:], wp_d.ap())
            nc.sync.dma_start(bq_sb[:], bq_d.ap())
            nc.sync.dma_start(bk_sb[:], bk_d.ap())
            nc.sync.dma_start(bvr_sb[:], bv_d.ap())
            nc.sync.dma_start(mask_sb[:], mask_d.ap().rearrange("b (t p) -> p b t", p=128))
            nc.vector.memset(ones1[:], 1.0)
            nc.vector.memset(ones64[:], 1.0)

            qT_sb = qpool.tile([128, T], bf16)   # rows 0:64 head0, 64:128 head1
            kT_sb = qpool.tile([128, T], bf16)
            v_sb = qpool.tile([128, T], bf16)    # tile g: [:, 128g:128g+128] = [tok, d]

            # --- phase 1: QKV projection -----------------------------------
            with (
                tc.tile_pool(name="hs", bufs=1) as hpool,
                tc.tile_pool(name="ps1", bufs=2, space="PSUM") as ps1,
            ):
                # v bias broadcast tile: ones1^T @ bv -> [128 tok, DC]
                bvp = ps1.tile([128, DC], f32, tag="vps")
                nc.tensor.matmul(bvp[:], lhsT=ones1[:], rhs=bvr_sb[:],
                                 start=True, stop=True)
                nc.vector.tensor_copy(bv_bc[:], bvp[:])

                hs_all = hpool.tile([128, KC, T], bf16)
                for blk in range(NBLK):
                    cols = slice(blk * 512, (blk + 1) * 512)
                    for k in range(KC):
                        nc.sync.dma_start(
                            hs_all[:, k, cols],
                            hsT_d.ap()[k * 128:(k + 1) * 128, cols])
                    q_ps = ps1.tile([128, 512], f32, tag="qkps")
                    for k in range(KC):
                        nc.tensor.matmul(q_ps[:], lhsT=wq_sb[:, k, :],
                                         rhs=hs_all[:, k, cols],
                                         start=(k == 0), stop=(k == KC - 1))
                    nc.vector.tensor_scalar_add(qT_sb[:, cols], q_ps[:], bq_sb[:, 0:1])
                    k_ps = ps1.tile([128, 512], f32, tag="qkps")
                    for k in range(KC):
                        nc.tensor.matmul(k_ps[:], lhsT=wk_sb[:, k, :],
                                         rhs=hs_all[:, k, cols],
                                         start=(k == 0), stop=(k == KC - 1))
                    nc.vector.tensor_scalar_add(kT_sb[:, cols], k_ps[:], bk_sb[:, 0:1])
                    for g4 in range(4):
                        g = blk * 4 + g4
                        tok = slice(g4 * 128, (g4 + 1) * 128)
                        v_ps = ps1.tile([128, DC], f32, tag="vps")
                        for k in range(KC):
                            nc.tensor.matmul(v_ps[:], lhsT=hs_all[:, k, cols][:, tok],
                                             rhs=wv_sb[:, k, :],
                                             start=(k == 0), stop=(k == KC - 1))
                        nc.vector.tensor_tensor(v_sb[:, g * 128:(g + 1) * 128],
                                                v_ps[:], bv_bc[:], op=ADD)

            # --- phase 2: attention + c_proj -------------------------------
            with (
                tc.tile_pool(name="probs", bufs=18) as ppool,
                tc.tile_pool(name="ctxn", bufs=2) as npool,
                tc.tile_pool(name="rec", bufs=2) as rpool,
                tc.tile_pool(name="ob", bufs=3) as opool,
                tc.tile_pool(name="ps_sc", bufs=2, space="PSUM") as scp,
                tc.tile_pool(name="ps_ctx", bufs=2, space="PSUM") as ctxp,
                tc.tile_pool(name="ps_se", bufs=2, space="PSUM") as sep,
            ):
                for b in range(B):
                    for sqb in range(SQB):
                        sq0 = b * S + sqb * 512
                        sq = slice(sq0, sq0 + 512)
                        # A: scores (row-tiled: 2 heads concurrently) + exp
                        probs = []
                        for skt in range(SKT):
                            sk0 = b * S + skt * 128
                            sk = slice(sk0, sk0 + 128)
                            sc_ps = scp.tile([128, 1024], f32, tag="sc")
                            nc.tensor.matmul(sc_ps[:, 0:512],
                                             lhsT=kT_sb[0:64, sk],
                                             rhs=qT_sb[0:64, sq],
                                             start=True, stop=True)
                            nc.tensor.matmul(sc_ps[:, 512:1024],
                                             lhsT=kT_sb[64:128, sk],
                                             rhs=qT_sb[64:128, sq],
                                             start=True, stop=True)
                            pr = ppool.tile([128, 1024], bf16, tag="pr")
                            nc.scalar.activation(pr[:], sc_ps[:], EXP,
                                                 bias=mask_sb[:, b, skt:skt + 1])
                            probs.append(pr)
                        # B: ctx + sumexp (col-tiled: 2 heads concurrently)
                        ctx_ps = ctxp.tile([128, 512], f32, tag="ctx")
                        se_ps = sep.tile([128, 512], f32, tag="se")
                        for skt in range(SKT):
                            g = b * SKT + skt
                            vt = v_sb[:, g * 128:(g + 1) * 128]
                            pr = probs[skt]
                            st, sp = (skt == 0), (skt == SKT - 1)
                            nc.tensor.matmul(ctx_ps[0:64, :], lhsT=vt[:, 0:64],
                                             rhs=pr[:, 0:512], start=st, stop=sp,
                                             skip_group_check=True)
                            nc.tensor.matmul(ctx_ps[64:128, :], lhsT=vt[:, 64:128],
                                             rhs=pr[:, 512:1024], start=st, stop=sp,
                                             skip_group_check=True)
                            nc.tensor.matmul(se_ps[0:64, :], lhsT=ones64[:],
                                             rhs=pr[:, 0:512], start=st, stop=sp,
                                             skip_group_check=True)
                            nc.tensor.matmul(se_ps[64:128, :], lhsT=ones64[:],
                                             rhs=pr[:, 512:1024], start=st, stop=sp,
                                             skip_group_check=True)
                        rec = rpool.tile([128, 512], f32, tag="rec")
                        nc.vector.reciprocal(rec[:], se_ps[:])
                        ctxn = npool.tile([128, 512], bf16, tag="ctxn")
                        nc.vector.tensor_tensor(ctxn[:], ctx_ps[:], rec[:], op=MULT)
                        # C: c_proj partial for the 4 token tiles of this block
                        for t4 in range(4):
                            tok = slice(t4 * 128, (t4 + 1) * 128)
                            op_ps = scp.tile([128, 1024], f32, tag="sc")
                            nc.tensor.matmul(op_ps[:, 0:512], lhsT=ctxn[:, tok],
                                             rhs=wp_sb[:, 0:512],
                                             start=True, stop=True)
                            nc.tensor.matmul(op_ps[:, 512:1024], lhsT=ctxn[:, tok],
                                             rhs=wp_sb[:, 512:1024],
                                             start=True, stop=True)
                            ob = opool.tile([128, 1024], f32, tag="ob")
                            nc.vector.tensor_copy(ob[:], op_ps[:])
                            nc.sync.dma_start(
                                out_d.ap()[sq0 + t4 * 128: sq0 + (t4 + 1) * 128, :],
                                ob[:])

    nc.compile()
    return nc


def _get_nc():
    if "nc" not in _CACHE:
        _CACHE["nc"] = _build_nc()
    return _CACHE["nc"]


def kernel(hidden_states, attention_mask, c_attn_w, c_attn_b, c_proj_w, c_proj_b):
    from concourse.bass_utils import run_bass_kernel_spmd

    bf16 = ml_dtypes.bfloat16
    hs = np.asarray(hidden_states, dtype=np.float32).reshape(T, H)
    hsT = np.ascontiguousarray(hs.T).astype(bf16)
    mask = np.ascontiguousarray(
        np.broadcast_to(
            np.asarray(attention_mask, dtype=np.float32).reshape(B, 1, 1, S)[:, 0, 0, :],
            (B, S),
        )
    )
    w = np.asarray(c_attn_w, dtype=np.float32)
    bqkv = np.asarray(c_attn_b, dtype=np.float32)
    wp_full = np.asarray(c_proj_w, dtype=np.float32)
    scale = 1.0 / np.sqrt(HD)

    in_maps = []
    for c in range(NCORES):
        lo, hi = c * DC, (c + 1) * DC
        in_maps.append({
            "hsT": hsT,
            "wq": np.ascontiguousarray(w[:, lo:hi] * scale).astype(bf16),
            "wk": np.ascontiguousarray(w[:, H + lo:H + hi]).astype(bf16),
            "wv": np.ascontiguousarray(w[:, 2 * H + lo:2 * H + hi]).astype(bf16),
            "wp": np.ascontiguousarray(wp_full[lo:hi, :]).astype(bf16),
            "bq": np.ascontiguousarray((bqkv[lo:hi] * scale).reshape(DC, 1)),
            "bk": np.ascontiguousarray(bqkv[H + lo:H + hi].reshape(DC, 1)),
            "bv": np.ascontiguousarray(bqkv[2 * H + lo:2 * H + hi].reshape(1, DC)).astype(bf16),
            "mask": mask,
        })

    res = run_bass_kernel_spmd(_get_nc(), in_maps, core_ids=list(range(NCORES)))
    _CACHE["last_result"] = res
    acc = np.zeros((T, H), dtype=np.float32)
    for c in range(NCORES):
        acc += res.results[c]["out"]
    acc += np.asarray(c_proj_b, dtype=np.float32)[None, :]
    return acc.reshape(B, S, H)
